# revision 22
# baseline (speedup 1.0000x reference)
# Trainium2 Bass kernel for nn_MixtureOfExperts_37237366456694.
#
# Reference computation (B=4096, D=1024, H1=H2=4096, D_OUT=1024, K=8, G_H=512):
#   U[:,k,:] = MLP_k(x)                      (3-layer ReLU MLP per expert)
#   g        = softmax(gate_MLP(x))          (B, K)
#   Q        = cayley(A); B_k = Q[:, k*128:(k+1)*128]
#   V[:,k,:] = U[:,k,:] @ (B_k B_k^T)
#   out      = (sum_k g[:,k] * V[:,k,:]) @ Wo + bo
#
# Key algebraic collapse (exact):
#   out[b] = sum_k g[b,k] * (U[b,k,:] @ w_k) + bo,   w_k = B_k B_k^T Wo
#          = sum_k g[b,k] * (h2_k[b] @ v_k + c_k) + bo
#   with v_k = W3_k @ w_k  (H2-vector), c_k = b3_k . w_k  (scalar).
# So the third expert layer + subspace projection + output head reduce to a
# matvec against a precomputed vector.  The tiny Cayley solve / folds are done
# on host in float64; the heavy compute (two 4096-wide matmul layers + gate MLP
# per expert) runs on device in fp16 with f32 PSUM accumulation.
#
# Sharding: expert-parallel — core k owns expert k (its W1/W2/b1/b2/v shards),
# gate weights replicated.  Each core returns
#   out_e = exp(own gate logit)        (1, B)
#   out_t = out_e * (h2 @ v_k)         (1, B)
# and the host combines:  out = (sum_k out_t + c_k*out_e) / (sum_k out_e) + bo
# (the softmax normalizer is just the cross-expert sum of exp-logits, i.e. the
# all-reduce term; doing the divide on host avoids any cross-core collective).
import os

import numpy as np

P = 128


class _Cfg:
    def __init__(self, B=4096, D=1024, H=4096, GH=512, NT=512, SLAB=1024):
        self.B, self.D, self.H, self.GH, self.NT, self.SLAB = B, D, H, GH, NT, SLAB
        self.DC = D // P      # d_in chunks
        self.HC = H // P      # hidden chunks (H1 == H2)
        self.GC = GH // P     # gate hidden chunks
        self.NSLAB = B // SLAB
        self.SN = SLAB // NT  # n-tiles per slab


def _build_nc(cfg):
    import concourse.bass as bass  # noqa: F401
    import concourse.mybir as mybir
    import concourse.tile as tile
    from concourse import bacc

    fp16 = mybir.dt.float16
    f32 = mybir.dt.float32
    Relu = mybir.ActivationFunctionType.Relu

    B, DC, HC, GC, NT, SLAB, SN, NSLAB = (
        cfg.B, cfg.DC, cfg.HC, cfg.GC, cfg.NT, cfg.SLAB, cfg.SN, cfg.NSLAB)
    GH = cfg.GH

    nc = bacc.Bacc(None, target_bir_lowering=False)
    # Everything partition-major so each SBUF tile loads with ONE dma_start
    # (multiple DMAs land on different queues and blow the per-instruction
    # sync-wait budget — ISA sync fields hold very few waits — of downstream
    # consumers).  All small constants are packed into two tensors (one per
    # dtype) so every ACT-bias / PE-lhsT const dependency is a single queue
    # semaphore that is observed once and never waited on again.
    # [p, dc, b] = x[b, dc*P+p]
    xTd = nc.dram_tensor("xT", (P, DC, B), fp16, kind="ExternalInput")
    # [hc, p, dc, m] = W1[dc*P+p, hc*P+m]
    W1d = nc.dram_tensor("W1", (HC, P, DC, P), fp16, kind="ExternalInput")
    # [fc, p, hc, m] = W2[hc*P+p, fc*P+m]
    W2d = nc.dram_tensor("W2", (HC, P, HC, P), fp16, kind="ExternalInput")
    # f32 consts: [b1 (HC) | b2 (HC) | bg1 (GC)]
    NF = 2 * HC + GC
    cfd = nc.dram_tensor("constf", (P, NF, 1), f32, kind="ExternalInput")
    # fp16 consts: [v (HC) | wg2 own-expert column (GC)]
    NH = HC + GC
    chd = nc.dram_tensor("consth", (P, NH, 1), fp16, kind="ExternalInput")
    # [p, dc, gh] = Wg1[dc*P+p, gh]
    Wg1d = nc.dram_tensor("Wg1", (P, DC, GH), fp16, kind="ExternalInput")
    out_s = nc.dram_tensor("out_s", (1, B), f32, kind="ExternalOutput")
    out_e = nc.dram_tensor("out_e", (1, B), f32, kind="ExternalOutput")

    with tile.TileContext(nc) as tc:
        with (
            tc.tile_pool(name="const", bufs=1) as const,
            tc.tile_pool(name="xp", bufs=2) as xp,
            tc.tile_pool(name="zp", bufs=2) as zp,
            tc.tile_pool(name="w1p", bufs=3) as w1p,
            tc.tile_pool(name="w2p", bufs=4) as w2p,
            tc.tile_pool(name="h1p", bufs=1) as h1p,
            tc.tile_pool(name="h2p", bufs=3) as h2p,
            tc.tile_pool(name="outp", bufs=4) as outp,
            tc.tile_pool(name="mmps", bufs=4, space="PSUM") as mmps,
            tc.tile_pool(name="vps", bufs=2, space="PSUM") as vps,
        ):
            # --- constants resident in SBUF for the whole kernel ---
            wg1_t = const.tile((P, DC, GH), fp16)
            nc.sync.dma_start(wg1_t[:], Wg1d[:])
            cf_t = const.tile((P, NF, 1), f32)
            nc.sync.dma_start(cf_t[:], cfd[:])
            ch_t = const.tile((P, NH, 1), fp16)
            nc.sync.dma_start(ch_t[:], chd[:])
            b1_t = cf_t[:, 0:HC, :]
            b2_t = cf_t[:, HC:2 * HC, :]
            bg1_t = cf_t[:, 2 * HC:2 * HC + GC, :]
            v_t = ch_t[:, 0:HC, :]
            wg2_t = ch_t[:, HC:HC + GC, :]

            for sl in range(NSLAB):
                s0 = sl * SLAB
                # --- x slab (transposed: d on partitions) ---
                xt = xp.tile((P, DC, SLAB), fp16, name="xt", tag="xt")
                nc.sync.dma_start(xt[:], xTd[:, :, s0:s0 + SLAB])

                # --- gate MLP for this slab ---
                z1 = zp.tile((P, GC, SLAB), fp16, name="z1", tag="z1")
                for gc in range(GC):
                    for n in range(SN):
                        ns = slice(n * NT, (n + 1) * NT)
                        ps = mmps.tile((P, NT), f32, name="ps_g", tag="mm")
                        for dc in range(DC):
                            nc.tensor.matmul(
                                ps, wg1_t[:, dc, gc * P:(gc + 1) * P],
                                xt[:, dc, ns],
                                start=(dc == 0), stop=(dc == DC - 1))
                        nc.scalar.activation(z1[:, gc, ns], ps, Relu,
                                             bias=bg1_t[:, gc, :])
                for n in range(SN):
                    ns = slice(n * NT, (n + 1) * NT)
                    lps = vps.tile((1, NT), f32, name="lps", tag="vec")
                    for gc in range(GC):
                        nc.tensor.matmul(lps, wg2_t[:, gc, :], z1[:, gc, ns],
                                         start=(gc == 0), stop=(gc == GC - 1))
                    # raw own-expert gate logit (no bias, no exp — host does
                    # e = exp(logit + bg2[k]) on these 4096 scalars)
                    lt = outp.tile((1, NT), f32, name="lt", tag="ot")
                    nc.vector.tensor_copy(lt[:], lps)
                    nc.sync.dma_start(out_e[0:1, s0 + n * NT:s0 + (n + 1) * NT],
                                      lt[:])

                # --- layer 1: h1 = relu(x @ W1 + b1), stored transposed ---
                h1 = h1p.tile((P, HC, SLAB), fp16, name="h1", tag="h1")
                for hc in range(HC):
                    w1s = w1p.tile((P, DC, P), fp16, name="w1s", tag="w1s")
                    nc.sync.dma_start(w1s[:], W1d[hc])
                    for n in range(SN):
                        ns = slice(n * NT, (n + 1) * NT)
                        ps = mmps.tile((P, NT), f32, name="ps_1", tag="mm")
                        for dc in range(DC):
                            nc.tensor.matmul(ps, w1s[:, dc, :], xt[:, dc, ns],
                                             start=(dc == 0),
                                             stop=(dc == DC - 1))
                        nc.scalar.activation(h1[:, hc, ns], ps, Relu,
                                             bias=b1_t[:, hc, :])

                # --- layer 2 + folded third layer (matvec with v) ---
                # sv[n] accumulates  sum_f v[f] * h2T[f, b]  across fc chunks.
                sv = [vps.tile((1, NT), f32, name=f"sv{n}", tag="vec")
                      for n in range(SN)]
                # Delay the sv matvec of chunk fc by one iteration so the PE
                # never waits on the ACT eviction of the h2 tile it consumes.
                pend = {}
                for fc in range(HC):
                    w2s = w2p.tile((P, HC, P), fp16, name="w2s", tag="w2s")
                    nc.sync.dma_start(w2s[:], W2d[fc])
                    for n in range(SN):
                        ns = slice(n * NT, (n + 1) * NT)
                        ps = mmps.tile((P, NT), f32, name="ps_2", tag="mm")
                        for hc in range(HC):
                            nc.tensor.matmul(ps, w2s[:, hc, :], h1[:, hc, ns],
                                             start=(hc == 0),
                                             stop=(hc == HC - 1))
                        h2t = h2p.tile((P, NT), fp16, name="h2t", tag="h2t")
                        nc.scalar.activation(h2t[:], ps, Relu,
                                             bias=b2_t[:, fc, :])
                        if fc > 0:
                            pfc, ph2 = pend[n]
                            nc.tensor.matmul(sv[n], v_t[:, pfc, :], ph2[:],
                                             start=(pfc == 0), stop=False)
                        pend[n] = (fc, h2t)
                for n in range(SN):
                    pfc, ph2 = pend[n]
                    nc.tensor.matmul(sv[n], v_t[:, pfc, :], ph2[:],
                                     start=False, stop=True)

                # raw s = h2 @ v: evict PSUM->SBUF on the (idle) vector
                # engine, then DMA out.  Gate weighting happens on host.
                for n in range(SN):
                    ot = outp.tile((1, NT), f32, name="ot", tag="ot")
                    nc.vector.tensor_copy(ot[:], sv[n])
                    nc.sync.dma_start(out_s[0:1, s0 + n * NT:s0 + (n + 1) * NT],
                                      ot[:])
    nc.compile()
    return nc


_STATE = {}
LAST_RESULTS = None  # BassKernelResults of the most recent device run
LAST_RUN_SECONDS = None  # wall time of the device-run call (excl. host prep)


def _get_nc(cfg):
    key = (cfg.B, cfg.D, cfg.H, cfg.GH, cfg.NT, cfg.SLAB)
    if key not in _STATE:
        _STATE[key] = _build_nc(cfg)
    return _STATE[key]


def _fold(W3, b3, A, Wo):
    """v_k = W3_k @ (B_k B_k^T Wo),  c_k = b3_k . (B_k B_k^T Wo)  in float64."""
    A64 = A.astype(np.float64)
    S = A64 - A64.T
    I = np.eye(A.shape[0])
    Q = np.linalg.solve(I - S, I + S)
    K = W3.shape[0]
    sub = Q.shape[1] // K
    Bq = Q.reshape(Q.shape[0], K, sub)                      # [d, k, s]
    coef = np.einsum('dks,d->ks', Bq, Wo[:, 0].astype(np.float64))
    w = np.einsum('dks,ks->kd', Bq, coef)                   # (K, dim)
    v = np.einsum('kfd,kd->kf', W3.astype(np.float64), w)   # (K, H2)
    c = np.einsum('kd,kd->k', b3.astype(np.float64), w)     # (K,)
    return v, c


def _prep_in_maps(cfg, x, W1, b1, W2, b2, v, Wg1, bg1, Wg2, bg2):
    fp16 = np.float16
    f32 = np.float32
    K = W1.shape[0]
    DC, HC, GC = cfg.DC, cfg.HC, cfg.GC

    # [p, dc, b]
    xT = np.ascontiguousarray(
        x.astype(fp16).T.reshape(DC, P, cfg.B).transpose(1, 0, 2))
    W1p = np.ascontiguousarray(
        W1.astype(fp16).reshape(K, DC, P, HC, P).transpose(0, 3, 2, 1, 4))
    W2p = np.ascontiguousarray(
        W2.astype(fp16).reshape(K, HC, P, HC, P).transpose(0, 3, 2, 1, 4))
    Wg1p = np.ascontiguousarray(
        Wg1.astype(fp16).reshape(DC, P, cfg.GH).transpose(1, 0, 2))

    # packed f32 consts (P, 2*HC+GC, 1): [b1 | b2 | bg1]
    NF = 2 * HC + GC
    constf = np.empty((K, P, NF, 1), f32)
    constf[:, :, 0:HC, 0] = b1.astype(f32).reshape(K, HC, P).transpose(0, 2, 1)
    constf[:, :, HC:2 * HC, 0] = (
        b2.astype(f32).reshape(K, HC, P).transpose(0, 2, 1))
    constf[:, :, 2 * HC:, 0] = bg1.astype(f32).reshape(GC, P).T[None]
    # packed fp16 consts (P, HC+GC, 1): [v | own-expert Wg2 column]
    NH = HC + GC
    consth = np.empty((K, P, NH, 1), fp16)
    consth[:, :, 0:HC, 0] = v.astype(fp16).reshape(K, HC, P).transpose(0, 2, 1)
    consth[:, :, HC:, 0] = (
        Wg2.astype(fp16).T.reshape(Wg2.shape[1], GC, P).transpose(0, 2, 1))[:K]

    in_maps = []
    for k in range(K):
        in_maps.append({
            "xT": xT,
            "W1": W1p[k],
            "W2": W2p[k],
            "constf": constf[k],
            "consth": consth[k],
            "Wg1": Wg1p,
        })
    return in_maps


def kernel(x, W1, b1, W2, b2, W3, b3, Wg1, bg1, Wg2, bg2, A, Wo, bo):
    global LAST_RESULTS, LAST_RUN_SECONDS
    import time

    from concourse.bass_utils import run_bass_kernel_spmd

    cfg = _Cfg(B=x.shape[0], D=x.shape[1], H=W1.shape[2], GH=Wg1.shape[1])
    K = W1.shape[0]

    v, c = _fold(W3, b3, A, Wo)
    in_maps = _prep_in_maps(cfg, x, W1, b1, W2, b2, v, Wg1, bg1, Wg2, bg2)
    nc = _get_nc(cfg)

    trace = bool(int(os.environ.get("MOE_TRACE", "0")))
    t0 = time.time()
    res = run_bass_kernel_spmd(
        nc, in_maps, core_ids=list(range(K)), trace=trace,
        trace_cores=list(range(K)) if trace else None,
    )
    LAST_RUN_SECONDS = time.time() - t0
    LAST_RESULTS = res

    s = np.stack([r["out_s"][0] for r in res.results]).astype(np.float64)
    logit = np.stack([r["out_e"][0] for r in res.results]).astype(np.float64)
    e = np.exp(logit + bg2.astype(np.float64)[:, None])
    num = (e * (s + c[:, None])).sum(axis=0)
    den = e.sum(axis=0)
    out = num / den + float(bo[0])
    return out.astype(np.float32)[:, None]


# revision 25
# speedup vs baseline: 24.1483x; 24.1483x over previous
# Trainium2 Bass kernel for nn_MixtureOfExperts_37237366456694.
#
# Reference computation (B=4096, D=1024, H1=H2=4096, D_OUT=1024, K=8, G_H=512):
#   U[:,k,:] = MLP_k(x)                      (3-layer ReLU MLP per expert)
#   g        = softmax(gate_MLP(x))          (B, K)
#   Q        = cayley(A); B_k = Q[:, k*128:(k+1)*128]
#   V[:,k,:] = U[:,k,:] @ (B_k B_k^T)
#   out      = (sum_k g[:,k] * V[:,k,:]) @ Wo + bo
#
# Key algebraic collapse (exact):
#   out[b] = sum_k g[b,k] * (U[b,k,:] @ w_k) + bo,   w_k = B_k B_k^T Wo
#          = sum_k g[b,k] * (h2_k[b] @ v_k + c_k) + bo
#   with v_k = W3_k @ w_k  (H2-vector), c_k = b3_k . w_k  (scalar).
# So the third expert layer + subspace projection + output head reduce to a
# matvec against a precomputed vector.  The tiny Cayley solve / folds are done
# on host in float64; the heavy compute (two 4096-wide matmul layers + gate MLP
# per expert) runs on device in fp16 with f32 PSUM accumulation.
#
# Sharding: expert-parallel — core k owns expert k (its W1/W2/b1/b2/v shards),
# gate weights replicated.  Each core returns
#   out_e = exp(own gate logit)        (1, B)
#   out_t = out_e * (h2 @ v_k)         (1, B)
# and the host combines:  out = (sum_k out_t + c_k*out_e) / (sum_k out_e) + bo
# (the softmax normalizer is just the cross-expert sum of exp-logits, i.e. the
# all-reduce term; doing the divide on host avoids any cross-core collective).
import os

import numpy as np

P = 128


class _Cfg:
    def __init__(self, B=4096, D=1024, H=4096, GH=512, NT=512, SLAB=1024,
                 reps=1):
        self.B, self.D, self.H, self.GH, self.NT, self.SLAB = B, D, H, GH, NT, SLAB
        self.DC = D // P      # d_in chunks
        self.HC = H // P      # hidden chunks (H1 == H2)
        self.GC = GH // P     # gate hidden chunks
        self.NSLAB = B // SLAB
        self.SN = SLAB // NT  # n-tiles per slab
        self.reps = reps      # >1 only for differential benchmarking


def _build_nc(cfg):
    import concourse.bass as bass  # noqa: F401
    import concourse.mybir as mybir
    import concourse.tile as tile
    from concourse import bacc

    fp16 = mybir.dt.float16
    f32 = mybir.dt.float32
    Relu = mybir.ActivationFunctionType.Relu

    B, DC, HC, GC, NT, SLAB, SN, NSLAB = (
        cfg.B, cfg.DC, cfg.HC, cfg.GC, cfg.NT, cfg.SLAB, cfg.SN, cfg.NSLAB)
    GH = cfg.GH

    nc = bacc.Bacc(None, target_bir_lowering=False)
    # Everything partition-major so each SBUF tile loads with ONE dma_start
    # (multiple DMAs land on different queues and blow the per-instruction
    # sync-wait budget — ISA sync fields hold very few waits — of downstream
    # consumers).  All small constants are packed into two tensors (one per
    # dtype) so every ACT-bias / PE-lhsT const dependency is a single queue
    # semaphore that is observed once and never waited on again.
    # [p, dc, b] = x[b, dc*P+p]
    xTd = nc.dram_tensor("xT", (P, DC, B), fp16, kind="ExternalInput")
    # [hc, p, dc, m] = W1[dc*P+p, hc*P+m]
    W1d = nc.dram_tensor("W1", (HC, P, DC, P), fp16, kind="ExternalInput")
    # [fc, p, hc, m] = W2[hc*P+p, fc*P+m]
    W2d = nc.dram_tensor("W2", (HC, P, HC, P), fp16, kind="ExternalInput")
    # f32 consts: [b1 (HC) | b2 (HC) | bg1 (GC)]
    NF = 2 * HC + GC
    cfd = nc.dram_tensor("constf", (P, NF, 1), f32, kind="ExternalInput")
    # fp16 consts: [v (HC) | wg2 own-expert column (GC)]
    NH = HC + GC
    chd = nc.dram_tensor("consth", (P, NH, 1), fp16, kind="ExternalInput")
    # [p, dc, gh] = Wg1[dc*P+p, gh]
    Wg1d = nc.dram_tensor("Wg1", (P, DC, GH), fp16, kind="ExternalInput")
    out_s = nc.dram_tensor("out_s", (1, B), f32, kind="ExternalOutput")
    out_e = nc.dram_tensor("out_e", (1, B), f32, kind="ExternalOutput")

    with tile.TileContext(nc) as tc:
        with (
            tc.tile_pool(name="const", bufs=1) as const,
            tc.tile_pool(name="xp", bufs=2) as xp,
            tc.tile_pool(name="zp", bufs=2) as zp,
            tc.tile_pool(name="w1p", bufs=3) as w1p,
            tc.tile_pool(name="w2p", bufs=4) as w2p,
            tc.tile_pool(name="h1p", bufs=1) as h1p,
            tc.tile_pool(name="h2p", bufs=3) as h2p,
            tc.tile_pool(name="outp", bufs=4) as outp,
            tc.tile_pool(name="mmps", bufs=4, space="PSUM") as mmps,
            tc.tile_pool(name="vps", bufs=2, space="PSUM") as vps,
        ):
            # --- constants resident in SBUF for the whole kernel ---
            wg1_t = const.tile((P, DC, GH), fp16)
            nc.sync.dma_start(wg1_t[:], Wg1d[:])
            cf_t = const.tile((P, NF, 1), f32)
            nc.sync.dma_start(cf_t[:], cfd[:])
            ch_t = const.tile((P, NH, 1), fp16)
            nc.sync.dma_start(ch_t[:], chd[:])
            b1_t = cf_t[:, 0:HC, :]
            b2_t = cf_t[:, HC:2 * HC, :]
            bg1_t = cf_t[:, 2 * HC:2 * HC + GC, :]
            v_t = ch_t[:, 0:HC, :]
            wg2_t = ch_t[:, HC:HC + GC, :]

            for sl in range(NSLAB * cfg.reps):
                sl = sl % NSLAB
                s0 = sl * SLAB
                # --- x slab (transposed: d on partitions) ---
                xt = xp.tile((P, DC, SLAB), fp16, name="xt", tag="xt")
                nc.sync.dma_start(xt[:], xTd[:, :, s0:s0 + SLAB])

                # --- gate MLP for this slab ---
                z1 = zp.tile((P, GC, SLAB), fp16, name="z1", tag="z1")
                for gc in range(GC):
                    for n in range(SN):
                        ns = slice(n * NT, (n + 1) * NT)
                        ps = mmps.tile((P, NT), f32, name="ps_g", tag="mm")
                        for dc in range(DC):
                            nc.tensor.matmul(
                                ps, wg1_t[:, dc, gc * P:(gc + 1) * P],
                                xt[:, dc, ns],
                                start=(dc == 0), stop=(dc == DC - 1))
                        nc.scalar.activation(z1[:, gc, ns], ps, Relu,
                                             bias=bg1_t[:, gc, :])
                for n in range(SN):
                    ns = slice(n * NT, (n + 1) * NT)
                    lps = vps.tile((1, NT), f32, name="lps", tag="vec")
                    for gc in range(GC):
                        nc.tensor.matmul(lps, wg2_t[:, gc, :], z1[:, gc, ns],
                                         start=(gc == 0), stop=(gc == GC - 1))
                    # raw own-expert gate logit (no bias, no exp — host does
                    # e = exp(logit + bg2[k]) on these 4096 scalars)
                    lt = outp.tile((1, NT), f32, name="lt", tag="ot")
                    nc.vector.tensor_copy(lt[:], lps)
                    nc.sync.dma_start(out_e[0:1, s0 + n * NT:s0 + (n + 1) * NT],
                                      lt[:])

                # --- layer 1: h1 = relu(x @ W1 + b1), stored transposed ---
                h1 = h1p.tile((P, HC, SLAB), fp16, name="h1", tag="h1")
                for hc in range(HC):
                    w1s = w1p.tile((P, DC, P), fp16, name="w1s", tag="w1s")
                    nc.sync.dma_start(w1s[:], W1d[hc])
                    for n in range(SN):
                        ns = slice(n * NT, (n + 1) * NT)
                        ps = mmps.tile((P, NT), f32, name="ps_1", tag="mm")
                        for dc in range(DC):
                            nc.tensor.matmul(ps, w1s[:, dc, :], xt[:, dc, ns],
                                             start=(dc == 0),
                                             stop=(dc == DC - 1))
                        nc.scalar.activation(h1[:, hc, ns], ps, Relu,
                                             bias=b1_t[:, hc, :])

                # --- layer 2 + folded third layer (matvec with v) ---
                # sv[n] accumulates  sum_f v[f] * h2T[f, b]  across fc chunks.
                sv = [vps.tile((1, NT), f32, name=f"sv{n}", tag="vec")
                      for n in range(SN)]
                # Delay the sv matvec of chunk fc by one iteration so the PE
                # never waits on the ACT eviction of the h2 tile it consumes.
                pend = {}
                for fc in range(HC):
                    w2s = w2p.tile((P, HC, P), fp16, name="w2s", tag="w2s")
                    nc.sync.dma_start(w2s[:], W2d[fc])
                    for n in range(SN):
                        ns = slice(n * NT, (n + 1) * NT)
                        ps = mmps.tile((P, NT), f32, name="ps_2", tag="mm")
                        for hc in range(HC):
                            nc.tensor.matmul(ps, w2s[:, hc, :], h1[:, hc, ns],
                                             start=(hc == 0),
                                             stop=(hc == HC - 1))
                        h2t = h2p.tile((P, NT), fp16, name="h2t", tag="h2t")
                        nc.scalar.activation(h2t[:], ps, Relu,
                                             bias=b2_t[:, fc, :])
                        if fc > 0:
                            pfc, ph2 = pend[n]
                            nc.tensor.matmul(sv[n], v_t[:, pfc, :], ph2[:],
                                             start=(pfc == 0), stop=False)
                        pend[n] = (fc, h2t)
                for n in range(SN):
                    pfc, ph2 = pend[n]
                    nc.tensor.matmul(sv[n], v_t[:, pfc, :], ph2[:],
                                     start=False, stop=True)

                # raw s = h2 @ v: evict PSUM->SBUF on the (idle) vector
                # engine, then DMA out.  Gate weighting happens on host.
                for n in range(SN):
                    ot = outp.tile((1, NT), f32, name="ot", tag="ot")
                    nc.vector.tensor_copy(ot[:], sv[n])
                    nc.sync.dma_start(out_s[0:1, s0 + n * NT:s0 + (n + 1) * NT],
                                      ot[:])
    nc.compile()
    return nc


_STATE = {}
LAST_RESULTS = None  # BassKernelResults of the most recent device run
LAST_RUN_SECONDS = None  # wall time of the device-run call (excl. host prep)


def _get_nc(cfg):
    key = (cfg.B, cfg.D, cfg.H, cfg.GH, cfg.NT, cfg.SLAB, cfg.reps)
    if key not in _STATE:
        _STATE[key] = _build_nc(cfg)
    return _STATE[key]


def _fold(W3, b3, A, Wo):
    """v_k = W3_k @ (B_k B_k^T Wo),  c_k = b3_k . (B_k B_k^T Wo)  in float64."""
    A64 = A.astype(np.float64)
    S = A64 - A64.T
    I = np.eye(A.shape[0])
    Q = np.linalg.solve(I - S, I + S)
    K = W3.shape[0]
    sub = Q.shape[1] // K
    Bq = Q.reshape(Q.shape[0], K, sub)                      # [d, k, s]
    coef = np.einsum('dks,d->ks', Bq, Wo[:, 0].astype(np.float64))
    w = np.einsum('dks,ks->kd', Bq, coef)                   # (K, dim)
    v = np.einsum('kfd,kd->kf', W3.astype(np.float64), w)   # (K, H2)
    c = np.einsum('kd,kd->k', b3.astype(np.float64), w)     # (K,)
    return v, c


def _prep_in_maps(cfg, x, W1, b1, W2, b2, v, Wg1, bg1, Wg2, bg2):
    fp16 = np.float16
    f32 = np.float32
    K = W1.shape[0]
    DC, HC, GC = cfg.DC, cfg.HC, cfg.GC

    # [p, dc, b]
    xT = np.ascontiguousarray(
        x.astype(fp16).T.reshape(DC, P, cfg.B).transpose(1, 0, 2))
    W1p = np.ascontiguousarray(
        W1.astype(fp16).reshape(K, DC, P, HC, P).transpose(0, 3, 2, 1, 4))
    W2p = np.ascontiguousarray(
        W2.astype(fp16).reshape(K, HC, P, HC, P).transpose(0, 3, 2, 1, 4))
    Wg1p = np.ascontiguousarray(
        Wg1.astype(fp16).reshape(DC, P, cfg.GH).transpose(1, 0, 2))

    # packed f32 consts (P, 2*HC+GC, 1): [b1 | b2 | bg1]
    NF = 2 * HC + GC
    constf = np.empty((K, P, NF, 1), f32)
    constf[:, :, 0:HC, 0] = b1.astype(f32).reshape(K, HC, P).transpose(0, 2, 1)
    constf[:, :, HC:2 * HC, 0] = (
        b2.astype(f32).reshape(K, HC, P).transpose(0, 2, 1))
    constf[:, :, 2 * HC:, 0] = bg1.astype(f32).reshape(GC, P).T[None]
    # packed fp16 consts (P, HC+GC, 1): [v | own-expert Wg2 column]
    NH = HC + GC
    consth = np.empty((K, P, NH, 1), fp16)
    consth[:, :, 0:HC, 0] = v.astype(fp16).reshape(K, HC, P).transpose(0, 2, 1)
    consth[:, :, HC:, 0] = (
        Wg2.astype(fp16).T.reshape(Wg2.shape[1], GC, P).transpose(0, 2, 1))[:K]

    in_maps = []
    for k in range(K):
        in_maps.append({
            "xT": xT,
            "W1": W1p[k],
            "W2": W2p[k],
            "constf": constf[k],
            "consth": consth[k],
            "Wg1": Wg1p,
        })
    return in_maps


def kernel(x, W1, b1, W2, b2, W3, b3, Wg1, bg1, Wg2, bg2, A, Wo, bo):
    global LAST_RESULTS, LAST_RUN_SECONDS
    import time

    from concourse.bass_utils import run_bass_kernel_spmd

    cfg = _Cfg(B=x.shape[0], D=x.shape[1], H=W1.shape[2], GH=Wg1.shape[1])
    K = W1.shape[0]

    v, c = _fold(W3, b3, A, Wo)
    in_maps = _prep_in_maps(cfg, x, W1, b1, W2, b2, v, Wg1, bg1, Wg2, bg2)
    nc = _get_nc(cfg)

    trace = bool(int(os.environ.get("MOE_TRACE", "0")))
    t0 = time.time()
    res = run_bass_kernel_spmd(
        nc, in_maps, core_ids=list(range(K)), trace=trace,
        trace_cores=list(range(K)) if trace else None,
    )
    LAST_RUN_SECONDS = time.time() - t0
    LAST_RESULTS = res

    s = np.stack([r["out_s"][0] for r in res.results]).astype(np.float64)
    logit = np.stack([r["out_e"][0] for r in res.results]).astype(np.float64)
    e = np.exp(logit + bg2.astype(np.float64)[:, None])
    num = (e * (s + c[:, None])).sum(axis=0)
    den = e.sum(axis=0)
    out = num / den + float(bo[0])
    return out.astype(np.float32)[:, None]
